# revision 17
# baseline (speedup 1.0000x reference)
"""Trainium2 Bass kernel for NT-Xent style contrastive loss (v4).

Math (B=4096, D=128, T=0.25), with z = row-normalized emb:
  S = z_i @ z_j^T   [B, B]
  loss = (1/2B) * sum_r [ -2*S[r,r]/T + ln(sum_c exp(S[r,c]/T))
                                      + ln(sum_c exp(S[c,r]/T)) ]
exp(S) is computed exactly once; row sums and column sums of it feed the
two ln branches.

Sharding: 2D. Core (rb, ch), rb = core//2, ch = core%2, owns the
[1024 rows x 2048 cols] block. All inputs are plain row slices (no host
rotation).

Orientation: S^T chunks [128 cols, rows]: stationary = scaled column
tile zcjT [d, 128 c], moving = normalized zT_i [d, 1024 r]. Both norm
factors are pre-applied to the operands (rows: z_i = ai/n_i; cols:
zcj = cj * 4/n_c, absorbing 1/T), so PSUM holds s/T directly and the
exp has a constant scale — ACT chunks span 2 c-tiles [128, 2048].

Reductions:
  - row-sum partials (branch a): Esum[c_p, r] = sum_tc exp-tile, built by
    DVE tensor_tensor_reduce ping-pong; 2 final mask-matmuls reduce the
    128 partitions -> psR [2, 512]
  - col sums (branch b): the same TTR's accum_out gives RUNNING column
    sums; host takes telescoping differences
  - diag (positives): DVE dot of z_i and own emb_j rows
Host sums partials across cores (pairs for row sums, quads for col sums),
takes ln, adds diag partials, divides by 2B.
"""

import numpy as np

B = 4096
D = 128
P = 128
NCORES = 8
RB = 1024                  # rows per core
CB = 2048                  # cols per core
RT = RB // P               # 8 row t-tiles
CT = CB // P               # 16 col t-tiles
TEMP = 0.25
LN4 = float(np.log(4.0))

_cache = {}


def _build_bass():
    import concourse.bass as bass
    import concourse.mybir as mybir
    import concourse.tile as tile
    from concourse.bass import broadcast_tensor_aps
    from concourse.tile_rust import add_dep_helper

    f32 = mybir.dt.float32
    bf16 = mybir.dt.bfloat16
    AF = mybir.ActivationFunctionType
    ALU = mybir.AluOpType
    AX = mybir.AxisListType

    nc = bass.Bass("TRN2")
    ai_d = nc.dram_tensor("emb_i_blk", [RB, D], f32, kind="ExternalInput")
    cj_d = nc.dram_tensor("emb_j_cols", [CB, D], f32, kind="ExternalInput")
    oj_d = nc.dram_tensor("emb_j_own", [RB, D], f32, kind="ExternalInput")
    out_cs = nc.dram_tensor("colsum", [P, CT + 1], f32, kind="ExternalOutput")
    out_rs = nc.dram_tensor("rowsum", [2, 512], f32, kind="ExternalOutput")

    ai_t = ai_d.rearrange("(t p) d -> p t d", p=P)   # [128, 8, 128]
    cj_t = cj_d.rearrange("(t p) d -> p t d", p=P)   # [128, 16, 128]
    oj_t = oj_d.rearrange("(t p) d -> p t d", p=P)   # [128, 8, 128]

    with tile.TileContext(nc) as tc:
        with (
            tc.tile_pool(name="persist", bufs=1) as persist,
            tc.tile_pool(name="scratch", bufs=4) as scratch,
            tc.tile_pool(name="ebuf", bufs=2) as ebuf,
            tc.tile_pool(name="psmain", bufs=2, space="PSUM") as psmain,
        ):
            cj = persist.tile([P, CT, D], bf16, tag="cj")
            zcj = persist.tile([P, CT, D], bf16, tag="zcj")
            zcjT = persist.tile([P, CT, D], bf16, tag="zcjT")
            ai = persist.tile([P, RT, D], bf16, tag="ai")
            z_i = persist.tile([P, RT, D], bf16, tag="z_i")
            zT_i = persist.tile([P, RT, D], bf16, tag="zT_i")
            oj = persist.tile([P, RT, D], bf16, tag="oj")
            mask = persist.tile([P, 16], bf16, tag="mask")
            zb = persist.tile([P, 1], f32, tag="zb")
            b_ln4 = persist.tile([P, 1], f32, tag="b_ln4")
            dummy_out = persist.tile([P, 16], bf16, tag="dummy_out")

            n2i = persist.tile([P, RT], f32, tag="n2i")
            invi = persist.tile([P, RT, 1], f32, tag="invi")
            n2o = persist.tile([P, RT], f32, tag="n2o")
            inv4o = persist.tile([P, RT], f32, tag="inv4o")
            n2c = persist.tile([P, CT], f32, tag="n2c")
            inv4c = persist.tile([P, CT, 1], f32, tag="inv4c")
            cs_sb = persist.tile([P, CT + 1], f32, tag="cs_sb")
            rs_sb = persist.tile([2, 512], f32, tag="rs_sb")
            ezero = persist.tile([P, RB], bf16, tag="ezero")
            esum = [
                persist.tile([P, RB], bf16, name="esum0", tag="esum0"),
                persist.tile([P, RB], bf16, name="esum1", tag="esum1"),
            ]

            nc.vector.memset(mask, 0.0)
            nc.vector.memset(mask[:, 8:9], 1.0)
            nc.vector.memset(zb, 0.0)
            nc.vector.memset(b_ln4, LN4)
            nc.vector.memset(ezero, 0.0)
            tblw = scratch.tile([P, 1], f32, tag="tblw")
            nc.scalar.activation(tblw, zb, AF.Ln, bias=b_ln4)

            # ---- loads (SWDGE cast fp32->bf16, one FIFO queue) ----
            nc.gpsimd.dma_start(out=ai, in_=ai_t)
            nc.gpsimd.dma_start(out=cj[:, 0:8, :], in_=cj_t[:, 0:8, :])
            nc.gpsimd.dma_start(out=cj[:, 8:16, :], in_=cj_t[:, 8:16, :])
            nc.gpsimd.dma_start(out=oj, in_=oj_t)

            # ---- norm stats: i then j groups (sq+reduce interleaved so
            # DVE fills the ACT ln/exp bubbles), broadcast-AP scales ----
            sqi = scratch.tile([P, RT, D], bf16, tag="sqi")
            nc.vector.tensor_mul(sqi, ai, ai)
            nc.vector.tensor_reduce(out=n2i, in_=sqi, axis=AX.X, op=ALU.add)
            lgi = scratch.tile([P, RT], f32, tag="lgi")
            nc.scalar.activation(lgi, n2i, AF.Ln, bias=zb)
            nc.scalar.activation(invi[:, :, 0], lgi, AF.Exp, scale=-0.5, bias=zb)

            def jstats(g):
                ts = slice(g * 8, (g + 1) * 8)
                sq = scratch.tile([P, 8, D], bf16, tag="sqj")
                nc.vector.tensor_mul(sq, cj[:, ts, :], cj[:, ts, :])
                nc.vector.tensor_reduce(out=n2c[:, ts], in_=sq, axis=AX.X, op=ALU.add)
                lgc = scratch.tile([P, 8], f32, tag="lgc")
                nc.scalar.activation(lgc, n2c[:, ts], AF.Ln, bias=zb)
                nc.scalar.activation(
                    inv4c[:, ts, 0], lgc, AF.Exp, scale=-0.5, bias=b_ln4
                )

            def jscale(g):
                ts = slice(g * 8, (g + 1) * 8)
                a_ap, b_ap = broadcast_tensor_aps(cj[:, ts, :], inv4c[:, ts, :])
                nc.vector.tensor_tensor(
                    out=zcj[:, ts, :], in0=a_ap, in1=b_ap, op=ALU.mult
                )

            jstats(0)
            a_ap, b_ap = broadcast_tensor_aps(ai[:, :, :], invi[:, :, :])
            nc.vector.tensor_tensor(out=z_i[:, :, :], in0=a_ap, in1=b_ap, op=ALU.mult)
            jscale(0)
            jstats(1)
            jscale(1)

            # ---- transposes ----
            dummy_inst = nc.sync.dma_start_transpose(
                out=dummy_out, in_=ai[0:16, 0, :]
            )
            tzi = nc.sync.dma_start_transpose(out=zT_i, in_=z_i)
            add_dep_helper(tzi.ins, dummy_inst.ins, False, "xpose after dummy")
            for lo, hi in ((0, 8), (8, 16)):
                tj = nc.sync.dma_start_transpose(
                    out=zcjT[:, lo:hi, :], in_=zcj[:, lo:hi, :]
                )
                add_dep_helper(tj.ins, dummy_inst.ins, False, "xpose after dummy")

            # ---- diag stats (GPSIMD — off the DVE ramp critical path) ----
            sqo = scratch.tile([P, RT, D], bf16, tag="sqo")
            nc.gpsimd.tensor_mul(sqo, oj, oj)
            nc.vector.tensor_reduce(out=n2o, in_=sqo, axis=AX.X, op=ALU.add)
            lgo = scratch.tile([P, RT], f32, tag="lgo")
            nc.scalar.activation(lgo, n2o, AF.Ln, bias=zb)
            nc.scalar.activation(inv4o, lgo, AF.Exp, scale=-0.5, bias=b_ln4)
            ddt = scratch.tile([P, RT, D], bf16, tag="ddt")
            nc.gpsimd.tensor_mul(ddt, z_i, oj)
            dvec = persist.tile([P, RT], f32, tag="dvec")
            nc.vector.tensor_reduce(out=dvec, in_=ddt, axis=AX.X, op=ALU.add)

            zTi_flat = zT_i.rearrange("p t d -> p (t d)")

            # ---- main loop: 8 chunks of 2 c-tiles ----
            for k in range(8):
                ps = psmain.tile([P, 2048], f32, tag="ps")
                for sub in range(2):
                    tcc = 2 * k + sub
                    for q in range(2):
                        nc.tensor.matmul(
                            ps[:, sub * 1024 + q * 512 : sub * 1024 + (q + 1) * 512],
                            zcjT[:, tcc, :],
                            zTi_flat[:, q * 512 : (q + 1) * 512],
                            start=True,
                            stop=True,
                        )
                eb = ebuf.tile([P, 2048], bf16, tag="eb")
                nc.scalar.activation(eb, ps, AF.Exp, bias=zb)
                for sub in range(2):
                    tcc = 2 * k + sub
                    prev = ezero if tcc == 0 else esum[(tcc - 1) % 2]
                    nc.vector.scalar_tensor_tensor(
                        out=esum[tcc % 2],
                        in0=eb[:, sub * RB : (sub + 1) * RB],
                        scalar=1.0,
                        in1=prev,
                        op0=ALU.mult,
                        op1=ALU.add,
                        accum_out=cs_sb[:, tcc : tcc + 1],
                    )

            # ---- tail: rowsum partials via 2 mask-matmuls on final Esum
            e_last = esum[(CT - 1) % 2]
            psR_full = psmain.tile([P, 2048], f32, tag="ps")
            psR = psR_full[0:2, 0:512]
            for q in range(2):
                nc.tensor.matmul(
                    psR,
                    mask[:, 8 - q : 10 - q],
                    e_last[:, q * 512 : (q + 1) * 512],
                    start=(q == 0),
                    stop=(q == 1),
                )

            dsc = scratch.tile([P, RT], f32, tag="dsc")
            nc.vector.tensor_mul(dsc, dvec, inv4o)
            nc.vector.tensor_scalar_mul(dsc, dsc, -2.0)
            nc.vector.tensor_reduce(
                out=cs_sb[:, CT : CT + 1], in_=dsc, axis=AX.X, op=ALU.add
            )
            nc.vector.tensor_copy(rs_sb, psR)

            nc.sync.dma_start(out=out_cs[:, :], in_=cs_sb)
            nc.sync.dma_start(out=out_rs[:, :], in_=rs_sb)

    return nc


def _split_multi_waits(bir: bytes) -> bytes:
    """The walrus build in this container accepts only ONE sync-wait per
    compute/DMA instruction. Tile emits up to three. Move all but one wait
    onto standalone EventSemaphore instructions inserted just before the
    offender on the same engine queue."""
    import json

    d = json.loads(bir)
    n_split = 0
    for fn in d["functions"]:
        for blk in fn["blocks"]:
            new_insts = []
            for ins in blk["instructions"]:
                si = ins.get("sync_info")
                waits = (si or {}).get("on_wait") or []
                if len(waits) > 1:
                    for w in waits[:-1]:
                        ev = {
                            "debug": ins.get("debug", 0),
                            "engine": ins["engine"],
                            "ins": [],
                            "outs": [],
                            "name": f"{ins['name']}_wsplit{n_split}",
                            "opcode": "EventSemaphore",
                            "sync_info": {"on_update": [], "on_wait": [w]},
                        }
                        n_split += 1
                        new_insts.append(ev)
                    si["on_wait"] = [waits[-1]]
                new_insts.append(ins)
            blk["instructions"] = new_insts
    return json.dumps(d).encode()


def kernel(emb_i: np.ndarray, emb_j: np.ndarray) -> np.ndarray:
    from concourse.bass_utils import run_bass_kernel_spmd

    if "nc" not in _cache:
        nc = _build_bass()
        fixed = _split_multi_waits(nc.to_json_bytes())
        nc.to_json_bytes = lambda: fixed
        _cache["nc"] = nc
    nc = _cache["nc"]

    emb_i = np.ascontiguousarray(emb_i, dtype=np.float32)
    emb_j = np.ascontiguousarray(emb_j, dtype=np.float32)
    in_maps = []
    for c in range(NCORES):
        rb, ch = c // 2, c % 2
        in_maps.append(
            {
                "emb_i_blk": emb_i[rb * RB : (rb + 1) * RB],
                "emb_j_cols": emb_j[ch * CB : (ch + 1) * CB],
                "emb_j_own": emb_j[rb * RB : (rb + 1) * RB],
            }
        )

    import os

    trace = bool(os.environ.get("KERNEL_TRACE"))
    res = run_bass_kernel_spmd(
        nc, in_maps, core_ids=list(range(NCORES)), trace=trace
    )
    _cache["last_res"] = res

    # host combine
    dtot = np.float64(0.0)
    cs_total = np.zeros(B, dtype=np.float64)
    rs_total = np.zeros(B, dtype=np.float64)
    for c, r in enumerate(res.results):
        rb, ch = c // 2, c % 2
        cs = r["colsum"].astype(np.float64)
        # cs[:, tc] are RUNNING column sums; telescoping differences
        run = cs[:, :CT]
        per_tile = np.diff(
            np.concatenate([np.zeros((P, 1)), run], axis=1), axis=1
        )
        # per_tile[p, tc] covers global col  ch*CB + tc*128 + p
        cs_total[ch * CB : (ch + 1) * CB] += per_tile.T.reshape(CB)
        dtot += np.float64(cs[:, CT].sum())
        rs_total[rb * RB : (rb + 1) * RB] += (
            r["rowsum"].reshape(RB).astype(np.float64)
        )
    total = dtot + np.log(rs_total).sum() + np.log(cs_total).sum()
    loss = total / (2 * B)
    return np.array(loss, dtype=np.float32)


# revision 19
# speedup vs baseline: 1.1924x; 1.1924x over previous
"""Trainium2 Bass kernel for NT-Xent style contrastive loss (v4).

Math (B=4096, D=128, T=0.25), with z = row-normalized emb:
  S = z_i @ z_j^T   [B, B]
  loss = (1/2B) * sum_r [ -2*S[r,r]/T + ln(sum_c exp(S[r,c]/T))
                                      + ln(sum_c exp(S[c,r]/T)) ]
exp(S) is computed exactly once; row sums and column sums of it feed the
two ln branches.

Sharding: 2D. Core (rb, ch), rb = core//2, ch = core%2, owns the
[1024 rows x 2048 cols] block. All inputs are plain row slices (no host
rotation).

Orientation: S^T chunks [128 cols, rows]: stationary = scaled column
tile zcjT [d, 128 c], moving = normalized zT_i [d, 1024 r]. Both norm
factors are pre-applied to the operands (rows: z_i = ai/n_i; cols:
zcj = cj * 4/n_c, absorbing 1/T), so PSUM holds s/T directly and the
exp has a constant scale — ACT chunks span 2 c-tiles [128, 2048].

Reductions:
  - row-sum partials (branch a): Esum[c_p, r] = sum_tc exp-tile, built by
    DVE tensor_tensor_reduce ping-pong; 2 final mask-matmuls reduce the
    128 partitions -> psR [2, 512]
  - col sums (branch b): the same TTR's accum_out gives RUNNING column
    sums; host takes telescoping differences
  - diag (positives): DVE dot of z_i and own emb_j rows
Host sums partials across cores (pairs for row sums, quads for col sums),
takes ln, adds diag partials, divides by 2B.
"""

import numpy as np

B = 4096
D = 128
P = 128
NCORES = 8
RB = 1024                  # rows per core
CB = 2048                  # cols per core
RT = RB // P               # 8 row t-tiles
CT = CB // P               # 16 col t-tiles
TEMP = 0.25
LN4 = float(np.log(4.0))

_cache = {}


def _build_bass():
    import concourse.bass as bass
    import concourse.mybir as mybir
    import concourse.tile as tile
    from concourse.bass import broadcast_tensor_aps
    from concourse.tile_rust import add_dep_helper

    f32 = mybir.dt.float32
    bf16 = mybir.dt.bfloat16
    AF = mybir.ActivationFunctionType
    ALU = mybir.AluOpType
    AX = mybir.AxisListType

    nc = bass.Bass("TRN2")
    ai_d = nc.dram_tensor("emb_i_blk", [RB, D], f32, kind="ExternalInput")
    cj_d = nc.dram_tensor("emb_j_cols", [CB, D], f32, kind="ExternalInput")
    oj_d = nc.dram_tensor("emb_j_own", [RB, D], f32, kind="ExternalInput")
    out_cs = nc.dram_tensor("colsum", [P, CT + 1], f32, kind="ExternalOutput")
    out_rs = nc.dram_tensor("rowsum", [2, 512], f32, kind="ExternalOutput")

    ai_t = ai_d.rearrange("(t p) d -> p t d", p=P)   # [128, 8, 128]
    cj_t = cj_d.rearrange("(t p) d -> p t d", p=P)   # [128, 16, 128]
    oj_t = oj_d.rearrange("(t p) d -> p t d", p=P)   # [128, 8, 128]

    with tile.TileContext(nc) as tc:
        with (
            tc.tile_pool(name="persist", bufs=1) as persist,
            tc.tile_pool(name="scratch", bufs=4) as scratch,
            tc.tile_pool(name="ebuf", bufs=2) as ebuf,
            tc.tile_pool(name="psmain", bufs=2, space="PSUM") as psmain,
        ):
            cj = persist.tile([P, CT, D], bf16, tag="cj")
            zcj = persist.tile([P, CT, D], bf16, tag="zcj")
            zcjT = persist.tile([P, CT, D], bf16, tag="zcjT")
            ai = persist.tile([P, RT, D], f32, tag="ai")
            z_i = persist.tile([P, RT, D], bf16, tag="z_i")
            zT_i = persist.tile([P, RT, D], bf16, tag="zT_i")
            oj = persist.tile([P, RT, D], bf16, tag="oj")
            mask = persist.tile([P, 16], bf16, tag="mask")
            zb = persist.tile([P, 1], f32, tag="zb")
            b_ln4 = persist.tile([P, 1], f32, tag="b_ln4")
            dummy_out = persist.tile([P, 16], bf16, tag="dummy_out")

            n2i = persist.tile([P, RT], f32, tag="n2i")
            invi = persist.tile([P, RT, 1], f32, tag="invi")
            n2o = persist.tile([P, RT], f32, tag="n2o")
            inv4o = persist.tile([P, RT], f32, tag="inv4o")
            n2c = persist.tile([P, CT], f32, tag="n2c")
            inv4c = persist.tile([P, CT, 1], f32, tag="inv4c")
            cs_sb = persist.tile([P, CT + 1], f32, tag="cs_sb")
            rs_sb = persist.tile([2, 512], f32, tag="rs_sb")
            ezero = persist.tile([P, RB], bf16, tag="ezero")
            esum = [
                persist.tile([P, RB], bf16, name="esum0", tag="esum0"),
                persist.tile([P, RB], bf16, name="esum1", tag="esum1"),
            ]

            nc.vector.memset(mask, 0.0)
            nc.vector.memset(mask[:, 8:9], 1.0)
            nc.vector.memset(zb, 0.0)
            nc.vector.memset(b_ln4, LN4)
            nc.vector.memset(ezero, 0.0)
            tblw = scratch.tile([P, 1], f32, tag="tblw")
            nc.scalar.activation(tblw, zb, AF.Ln, bias=b_ln4)

            # ---- loads: ai fp32 via HWDGE (parallel queue); cj in
            # 4+4+8-tile chunks + oj via the SWDGE cast queue ----
            nc.sync.dma_start(out=ai, in_=ai_t)
            nc.gpsimd.dma_start(out=cj[:, 0:4, :], in_=cj_t[:, 0:4, :])
            nc.gpsimd.dma_start(out=cj[:, 4:8, :], in_=cj_t[:, 4:8, :])
            nc.gpsimd.dma_start(out=cj[:, 8:16, :], in_=cj_t[:, 8:16, :])
            nc.gpsimd.dma_start(out=oj, in_=oj_t)

            # ---- norm stats: i then j groups (sq+reduce interleaved so
            # DVE fills the ACT ln/exp bubbles), broadcast-AP scales ----
            sqi = scratch.tile([P, RT, D], bf16, tag="sqi")
            nc.vector.tensor_mul(sqi, ai, ai)
            nc.vector.tensor_reduce(out=n2i, in_=sqi, axis=AX.X, op=ALU.add)
            lgi = scratch.tile([P, RT], f32, tag="lgi")
            nc.scalar.activation(lgi, n2i, AF.Ln, bias=zb)
            nc.scalar.activation(invi[:, :, 0], lgi, AF.Exp, scale=-0.5, bias=zb)

            JG = ((0, 4), (4, 8), (8, 16))

            def jstats(g):
                lo, hi = JG[g]
                ts = slice(lo, hi)
                sq = scratch.tile([P, hi - lo, D], bf16, name=f"sqj{g}", tag=f"sqj{g}")
                nc.vector.tensor_mul(sq, cj[:, ts, :], cj[:, ts, :])
                nc.vector.tensor_reduce(out=n2c[:, ts], in_=sq, axis=AX.X, op=ALU.add)
                lgc = scratch.tile([P, hi - lo], f32, name=f"lgc{g}", tag=f"lgc{g}")
                nc.scalar.activation(lgc, n2c[:, ts], AF.Ln, bias=zb)
                nc.scalar.activation(
                    inv4c[:, ts, 0], lgc, AF.Exp, scale=-0.5, bias=b_ln4
                )

            def jscale(g):
                lo, hi = JG[g]
                ts = slice(lo, hi)
                a_ap, b_ap = broadcast_tensor_aps(cj[:, ts, :], inv4c[:, ts, :])
                nc.vector.tensor_tensor(
                    out=zcj[:, ts, :], in0=a_ap, in1=b_ap, op=ALU.mult
                )

            a_ap, b_ap = broadcast_tensor_aps(ai[:, :, :], invi[:, :, :])
            nc.vector.tensor_tensor(out=z_i[:, :, :], in0=a_ap, in1=b_ap, op=ALU.mult)
            for g in range(3):
                jstats(g)
                jscale(g)

            # ---- transposes ----
            dummy_inst = nc.sync.dma_start_transpose(
                out=dummy_out, in_=cj[0:16, 0, :]
            )
            tzi = nc.sync.dma_start_transpose(out=zT_i, in_=z_i)
            add_dep_helper(tzi.ins, dummy_inst.ins, False, "xpose after dummy")
            for lo, hi in ((0, 4), (4, 8), (8, 16)):
                tj = nc.sync.dma_start_transpose(
                    out=zcjT[:, lo:hi, :], in_=zcj[:, lo:hi, :]
                )
                add_dep_helper(tj.ins, dummy_inst.ins, False, "xpose after dummy")

            # ---- diag stats (GPSIMD — off the DVE ramp critical path) ----
            sqo = scratch.tile([P, RT, D], bf16, tag="sqo")
            nc.gpsimd.tensor_mul(sqo, oj, oj)
            nc.vector.tensor_reduce(out=n2o, in_=sqo, axis=AX.X, op=ALU.add)
            lgo = scratch.tile([P, RT], f32, tag="lgo")
            nc.scalar.activation(lgo, n2o, AF.Ln, bias=zb)
            nc.scalar.activation(inv4o, lgo, AF.Exp, scale=-0.5, bias=b_ln4)
            ddt = scratch.tile([P, RT, D], bf16, tag="ddt")
            nc.gpsimd.tensor_mul(ddt, z_i, oj)
            dvec = persist.tile([P, RT], f32, tag="dvec")
            nc.vector.tensor_reduce(out=dvec, in_=ddt, axis=AX.X, op=ALU.add)

            zTi_flat = zT_i.rearrange("p t d -> p (t d)")

            # ---- main loop: 8 chunks of 2 c-tiles ----
            for k in range(8):
                ps = psmain.tile([P, 2048], f32, tag="ps")
                for sub in range(2):
                    tcc = 2 * k + sub
                    for q in range(2):
                        nc.tensor.matmul(
                            ps[:, sub * 1024 + q * 512 : sub * 1024 + (q + 1) * 512],
                            zcjT[:, tcc, :],
                            zTi_flat[:, q * 512 : (q + 1) * 512],
                            start=True,
                            stop=True,
                        )
                eb = ebuf.tile([P, 2048], bf16, tag="eb")
                nc.scalar.activation(eb, ps, AF.Exp, bias=zb)
                for sub in range(2):
                    tcc = 2 * k + sub
                    prev = ezero if tcc == 0 else esum[(tcc - 1) % 2]
                    nc.vector.scalar_tensor_tensor(
                        out=esum[tcc % 2],
                        in0=eb[:, sub * RB : (sub + 1) * RB],
                        scalar=1.0,
                        in1=prev,
                        op0=ALU.mult,
                        op1=ALU.add,
                        accum_out=cs_sb[:, tcc : tcc + 1],
                    )

            # ---- tail: rowsum partials via 2 mask-matmuls on final Esum
            e_last = esum[(CT - 1) % 2]
            psR_full = psmain.tile([P, 2048], f32, tag="ps")
            psR = psR_full[0:2, 0:512]
            for q in range(2):
                nc.tensor.matmul(
                    psR,
                    mask[:, 8 - q : 10 - q],
                    e_last[:, q * 512 : (q + 1) * 512],
                    start=(q == 0),
                    stop=(q == 1),
                )

            dsc = scratch.tile([P, RT], f32, tag="dsc")
            nc.vector.tensor_mul(dsc, dvec, inv4o)
            nc.vector.tensor_scalar_mul(dsc, dsc, -2.0)
            nc.vector.tensor_reduce(
                out=cs_sb[:, CT : CT + 1], in_=dsc, axis=AX.X, op=ALU.add
            )
            nc.vector.tensor_copy(rs_sb, psR)

            nc.sync.dma_start(out=out_cs[:, :], in_=cs_sb)
            nc.sync.dma_start(out=out_rs[:, :], in_=rs_sb)

    return nc


def _split_multi_waits(bir: bytes) -> bytes:
    """The walrus build in this container accepts only ONE sync-wait per
    compute/DMA instruction. Tile emits up to three. Move all but one wait
    onto standalone EventSemaphore instructions inserted just before the
    offender on the same engine queue."""
    import json

    d = json.loads(bir)
    n_split = 0
    for fn in d["functions"]:
        for blk in fn["blocks"]:
            new_insts = []
            for ins in blk["instructions"]:
                si = ins.get("sync_info")
                waits = (si or {}).get("on_wait") or []
                if len(waits) > 1:
                    for w in waits[:-1]:
                        ev = {
                            "debug": ins.get("debug", 0),
                            "engine": ins["engine"],
                            "ins": [],
                            "outs": [],
                            "name": f"{ins['name']}_wsplit{n_split}",
                            "opcode": "EventSemaphore",
                            "sync_info": {"on_update": [], "on_wait": [w]},
                        }
                        n_split += 1
                        new_insts.append(ev)
                    si["on_wait"] = [waits[-1]]
                new_insts.append(ins)
            blk["instructions"] = new_insts
    return json.dumps(d).encode()


def kernel(emb_i: np.ndarray, emb_j: np.ndarray) -> np.ndarray:
    from concourse.bass_utils import run_bass_kernel_spmd

    if "nc" not in _cache:
        nc = _build_bass()
        fixed = _split_multi_waits(nc.to_json_bytes())
        nc.to_json_bytes = lambda: fixed
        _cache["nc"] = nc
    nc = _cache["nc"]

    emb_i = np.ascontiguousarray(emb_i, dtype=np.float32)
    emb_j = np.ascontiguousarray(emb_j, dtype=np.float32)
    in_maps = []
    for c in range(NCORES):
        rb, ch = c // 2, c % 2
        in_maps.append(
            {
                "emb_i_blk": emb_i[rb * RB : (rb + 1) * RB],
                "emb_j_cols": emb_j[ch * CB : (ch + 1) * CB],
                "emb_j_own": emb_j[rb * RB : (rb + 1) * RB],
            }
        )

    import os

    trace = bool(os.environ.get("KERNEL_TRACE"))
    res = run_bass_kernel_spmd(
        nc, in_maps, core_ids=list(range(NCORES)), trace=trace
    )
    _cache["last_res"] = res

    # host combine
    dtot = np.float64(0.0)
    cs_total = np.zeros(B, dtype=np.float64)
    rs_total = np.zeros(B, dtype=np.float64)
    for c, r in enumerate(res.results):
        rb, ch = c // 2, c % 2
        cs = r["colsum"].astype(np.float64)
        # cs[:, tc] are RUNNING column sums; telescoping differences
        run = cs[:, :CT]
        per_tile = np.diff(
            np.concatenate([np.zeros((P, 1)), run], axis=1), axis=1
        )
        # per_tile[p, tc] covers global col  ch*CB + tc*128 + p
        cs_total[ch * CB : (ch + 1) * CB] += per_tile.T.reshape(CB)
        dtot += np.float64(cs[:, CT].sum())
        rs_total[rb * RB : (rb + 1) * RB] += (
            r["rowsum"].reshape(RB).astype(np.float64)
        )
    total = dtot + np.log(rs_total).sum() + np.log(cs_total).sum()
    loss = total / (2 * B)
    return np.array(loss, dtype=np.float32)


# revision 20
# speedup vs baseline: 1.1963x; 1.0033x over previous
"""Trainium2 Bass kernel for NT-Xent style contrastive loss (v4).

Math (B=4096, D=128, T=0.25), with z = row-normalized emb:
  S = z_i @ z_j^T   [B, B]
  loss = (1/2B) * sum_r [ -2*S[r,r]/T + ln(sum_c exp(S[r,c]/T))
                                      + ln(sum_c exp(S[c,r]/T)) ]
exp(S) is computed exactly once; row sums and column sums of it feed the
two ln branches.

Sharding: 2D. Core (rb, ch), rb = core//2, ch = core%2, owns the
[1024 rows x 2048 cols] block. All inputs are plain row slices (no host
rotation).

Orientation: S^T chunks [128 cols, rows]: stationary = scaled column
tile zcjT [d, 128 c], moving = normalized zT_i [d, 1024 r]. Both norm
factors are pre-applied to the operands (rows: z_i = ai/n_i; cols:
zcj = cj * 4/n_c, absorbing 1/T), so PSUM holds s/T directly and the
exp has a constant scale — ACT chunks span 2 c-tiles [128, 2048].

Reductions:
  - row-sum partials (branch a): Esum[c_p, r] = sum_tc exp-tile, built by
    DVE tensor_tensor_reduce ping-pong; 2 final mask-matmuls reduce the
    128 partitions -> psR [2, 512]
  - col sums (branch b): the same TTR's accum_out gives RUNNING column
    sums; host takes telescoping differences
  - diag (positives): DVE dot of z_i and own emb_j rows
Host sums partials across cores (pairs for row sums, quads for col sums),
takes ln, adds diag partials, divides by 2B.
"""

import numpy as np

B = 4096
D = 128
P = 128
NCORES = 8
RB = 1024                  # rows per core
CB = 2048                  # cols per core
RT = RB // P               # 8 row t-tiles
CT = CB // P               # 16 col t-tiles
TEMP = 0.25
LN4 = float(np.log(4.0))

_cache = {}


def _build_bass():
    import concourse.bass as bass
    import concourse.mybir as mybir
    import concourse.tile as tile
    from concourse.bass import broadcast_tensor_aps
    from concourse.tile_rust import add_dep_helper

    f32 = mybir.dt.float32
    bf16 = mybir.dt.bfloat16
    AF = mybir.ActivationFunctionType
    ALU = mybir.AluOpType
    AX = mybir.AxisListType

    nc = bass.Bass("TRN2")
    ai_d = nc.dram_tensor("emb_i_blk", [RB, D], f32, kind="ExternalInput")
    cj_d = nc.dram_tensor("emb_j_cols", [CB, D], f32, kind="ExternalInput")
    oj_d = nc.dram_tensor("emb_j_own", [RB, D], f32, kind="ExternalInput")
    out_cs = nc.dram_tensor("colsum", [P, CT + 1], f32, kind="ExternalOutput")
    out_rs = nc.dram_tensor("rowsum", [2, 512], f32, kind="ExternalOutput")

    ai_t = ai_d.rearrange("(t p) d -> p t d", p=P)   # [128, 8, 128]
    cj_t = cj_d.rearrange("(t p) d -> p t d", p=P)   # [128, 16, 128]
    oj_t = oj_d.rearrange("(t p) d -> p t d", p=P)   # [128, 8, 128]

    with tile.TileContext(nc) as tc:
        with (
            tc.tile_pool(name="persist", bufs=1) as persist,
            tc.tile_pool(name="scratch", bufs=4) as scratch,
            tc.tile_pool(name="ebuf", bufs=2) as ebuf,
            tc.tile_pool(name="psmain", bufs=2, space="PSUM") as psmain,
        ):
            cj = persist.tile([P, CT, D], bf16, tag="cj")
            zcj = persist.tile([P, CT, D], bf16, tag="zcj")
            zcjT = persist.tile([P, CT, D], bf16, tag="zcjT")
            ai = persist.tile([P, RT, D], f32, tag="ai")
            z_i = persist.tile([P, RT, D], bf16, tag="z_i")
            zT_i = persist.tile([P, RT, D], bf16, tag="zT_i")
            oj = persist.tile([P, RT, D], f32, tag="oj")
            mask = persist.tile([P, 16], bf16, tag="mask")
            zb = persist.tile([P, 1], f32, tag="zb")
            b_ln4 = persist.tile([P, 1], f32, tag="b_ln4")
            dummy_out = persist.tile([P, 16], bf16, tag="dummy_out")

            n2i = persist.tile([P, RT], f32, tag="n2i")
            invi = persist.tile([P, RT, 1], f32, tag="invi")
            n2o = persist.tile([P, RT], f32, tag="n2o")
            inv4o = persist.tile([P, RT], f32, tag="inv4o")
            n2c = persist.tile([P, CT], f32, tag="n2c")
            inv4c = persist.tile([P, CT, 1], f32, tag="inv4c")
            cs_sb = persist.tile([P, CT + 1], f32, tag="cs_sb")
            rs_sb = persist.tile([2, 512], f32, tag="rs_sb")
            ezero = persist.tile([P, RB], bf16, tag="ezero")
            esum = [
                persist.tile([P, RB], bf16, name="esum0", tag="esum0"),
                persist.tile([P, RB], bf16, name="esum1", tag="esum1"),
            ]

            nc.vector.memset(mask, 0.0)
            nc.vector.memset(mask[:, 8:9], 1.0)
            nc.vector.memset(zb, 0.0)
            nc.vector.memset(b_ln4, LN4)
            nc.vector.memset(ezero, 0.0)
            tblw = scratch.tile([P, 1], f32, tag="tblw")
            nc.scalar.activation(tblw, zb, AF.Ln, bias=b_ln4)

            # ---- loads: ai fp32 via HWDGE (parallel queue); cj in
            # 4+4+8-tile chunks + oj via the SWDGE cast queue ----
            nc.sync.dma_start(out=ai, in_=ai_t)
            nc.sync.dma_start(out=oj, in_=oj_t)
            nc.gpsimd.dma_start(out=cj[:, 0:4, :], in_=cj_t[:, 0:4, :])
            nc.gpsimd.dma_start(out=cj[:, 4:8, :], in_=cj_t[:, 4:8, :])
            nc.gpsimd.dma_start(out=cj[:, 8:16, :], in_=cj_t[:, 8:16, :])

            # ---- norm stats: i then j groups (sq+reduce interleaved so
            # DVE fills the ACT ln/exp bubbles), broadcast-AP scales ----
            sqi = scratch.tile([P, RT, D], bf16, tag="sqi")
            nc.vector.tensor_mul(sqi, ai, ai)
            nc.vector.tensor_reduce(out=n2i, in_=sqi, axis=AX.X, op=ALU.add)
            lgi = scratch.tile([P, RT], f32, tag="lgi")
            nc.scalar.activation(lgi, n2i, AF.Ln, bias=zb)
            nc.scalar.activation(invi[:, :, 0], lgi, AF.Exp, scale=-0.5, bias=zb)

            JG = ((0, 4), (4, 16))

            def jstats(g):
                lo, hi = JG[g]
                ts = slice(lo, hi)
                sq = scratch.tile([P, hi - lo, D], bf16, name=f"sqj{g}", tag=f"sqj{g}")
                nc.vector.tensor_mul(sq, cj[:, ts, :], cj[:, ts, :])
                nc.vector.tensor_reduce(out=n2c[:, ts], in_=sq, axis=AX.X, op=ALU.add)
                lgc = scratch.tile([P, hi - lo], f32, name=f"lgc{g}", tag=f"lgc{g}")
                nc.scalar.activation(lgc, n2c[:, ts], AF.Ln, bias=zb)
                nc.scalar.activation(
                    inv4c[:, ts, 0], lgc, AF.Exp, scale=-0.5, bias=b_ln4
                )

            def jscale(g):
                lo, hi = JG[g]
                ts = slice(lo, hi)
                a_ap, b_ap = broadcast_tensor_aps(cj[:, ts, :], inv4c[:, ts, :])
                nc.vector.tensor_tensor(
                    out=zcj[:, ts, :], in0=a_ap, in1=b_ap, op=ALU.mult
                )

            a_ap, b_ap = broadcast_tensor_aps(ai[:, :, :], invi[:, :, :])
            nc.vector.tensor_tensor(out=z_i[:, :, :], in0=a_ap, in1=b_ap, op=ALU.mult)
            for g in range(2):
                jstats(g)
                jscale(g)

            # ---- transposes ----
            dummy_inst = nc.sync.dma_start_transpose(
                out=dummy_out, in_=cj[0:16, 0, :]
            )
            tzi = nc.sync.dma_start_transpose(out=zT_i, in_=z_i)
            add_dep_helper(tzi.ins, dummy_inst.ins, False, "xpose after dummy")
            for lo, hi in ((0, 4), (4, 16)):
                tj = nc.sync.dma_start_transpose(
                    out=zcjT[:, lo:hi, :], in_=zcj[:, lo:hi, :]
                )
                add_dep_helper(tj.ins, dummy_inst.ins, False, "xpose after dummy")

            # ---- diag stats (GPSIMD — off the DVE ramp critical path) ----
            sqo = scratch.tile([P, RT, D], bf16, tag="sqo")
            nc.gpsimd.tensor_mul(sqo, oj, oj)
            nc.vector.tensor_reduce(out=n2o, in_=sqo, axis=AX.X, op=ALU.add)
            lgo = scratch.tile([P, RT], f32, tag="lgo")
            nc.scalar.activation(lgo, n2o, AF.Ln, bias=zb)
            nc.scalar.activation(inv4o, lgo, AF.Exp, scale=-0.5, bias=b_ln4)
            ddt = scratch.tile([P, RT, D], bf16, tag="ddt")
            nc.gpsimd.tensor_mul(ddt, z_i, oj)
            dvec = persist.tile([P, RT], f32, tag="dvec")
            nc.vector.tensor_reduce(out=dvec, in_=ddt, axis=AX.X, op=ALU.add)

            zTi_flat = zT_i.rearrange("p t d -> p (t d)")

            # ---- main loop: 8 chunks of 2 c-tiles ----
            for k in range(8):
                ps = psmain.tile([P, 2048], f32, tag="ps")
                for sub in range(2):
                    tcc = 2 * k + sub
                    for q in range(2):
                        nc.tensor.matmul(
                            ps[:, sub * 1024 + q * 512 : sub * 1024 + (q + 1) * 512],
                            zcjT[:, tcc, :],
                            zTi_flat[:, q * 512 : (q + 1) * 512],
                            start=True,
                            stop=True,
                        )
                eb = ebuf.tile([P, 2048], bf16, tag="eb")
                nc.scalar.activation(eb, ps, AF.Exp, bias=zb)
                for sub in range(2):
                    tcc = 2 * k + sub
                    prev = ezero if tcc == 0 else esum[(tcc - 1) % 2]
                    nc.vector.scalar_tensor_tensor(
                        out=esum[tcc % 2],
                        in0=eb[:, sub * RB : (sub + 1) * RB],
                        scalar=1.0,
                        in1=prev,
                        op0=ALU.mult,
                        op1=ALU.add,
                        accum_out=cs_sb[:, tcc : tcc + 1],
                    )

            # ---- tail: rowsum partials via 2 mask-matmuls on final Esum
            e_last = esum[(CT - 1) % 2]
            psR_full = psmain.tile([P, 2048], f32, tag="ps")
            psR = psR_full[0:2, 0:512]
            for q in range(2):
                nc.tensor.matmul(
                    psR,
                    mask[:, 8 - q : 10 - q],
                    e_last[:, q * 512 : (q + 1) * 512],
                    start=(q == 0),
                    stop=(q == 1),
                )

            dsc = scratch.tile([P, RT], f32, tag="dsc")
            nc.vector.tensor_mul(dsc, dvec, inv4o)
            nc.vector.tensor_scalar_mul(dsc, dsc, -2.0)
            nc.vector.tensor_reduce(
                out=cs_sb[:, CT : CT + 1], in_=dsc, axis=AX.X, op=ALU.add
            )
            nc.vector.tensor_copy(rs_sb, psR)

            nc.sync.dma_start(out=out_cs[:, :], in_=cs_sb)
            nc.sync.dma_start(out=out_rs[:, :], in_=rs_sb)

    return nc


def _split_multi_waits(bir: bytes) -> bytes:
    """The walrus build in this container accepts only ONE sync-wait per
    compute/DMA instruction. Tile emits up to three. Move all but one wait
    onto standalone EventSemaphore instructions inserted just before the
    offender on the same engine queue."""
    import json

    d = json.loads(bir)
    n_split = 0
    for fn in d["functions"]:
        for blk in fn["blocks"]:
            new_insts = []
            for ins in blk["instructions"]:
                si = ins.get("sync_info")
                waits = (si or {}).get("on_wait") or []
                if len(waits) > 1:
                    for w in waits[:-1]:
                        ev = {
                            "debug": ins.get("debug", 0),
                            "engine": ins["engine"],
                            "ins": [],
                            "outs": [],
                            "name": f"{ins['name']}_wsplit{n_split}",
                            "opcode": "EventSemaphore",
                            "sync_info": {"on_update": [], "on_wait": [w]},
                        }
                        n_split += 1
                        new_insts.append(ev)
                    si["on_wait"] = [waits[-1]]
                new_insts.append(ins)
            blk["instructions"] = new_insts
    return json.dumps(d).encode()


def kernel(emb_i: np.ndarray, emb_j: np.ndarray) -> np.ndarray:
    from concourse.bass_utils import run_bass_kernel_spmd

    if "nc" not in _cache:
        nc = _build_bass()
        fixed = _split_multi_waits(nc.to_json_bytes())
        nc.to_json_bytes = lambda: fixed
        _cache["nc"] = nc
    nc = _cache["nc"]

    emb_i = np.ascontiguousarray(emb_i, dtype=np.float32)
    emb_j = np.ascontiguousarray(emb_j, dtype=np.float32)
    in_maps = []
    for c in range(NCORES):
        rb, ch = c // 2, c % 2
        in_maps.append(
            {
                "emb_i_blk": emb_i[rb * RB : (rb + 1) * RB],
                "emb_j_cols": emb_j[ch * CB : (ch + 1) * CB],
                "emb_j_own": emb_j[rb * RB : (rb + 1) * RB],
            }
        )

    import os

    trace = bool(os.environ.get("KERNEL_TRACE"))
    res = run_bass_kernel_spmd(
        nc, in_maps, core_ids=list(range(NCORES)), trace=trace
    )
    _cache["last_res"] = res

    # host combine
    dtot = np.float64(0.0)
    cs_total = np.zeros(B, dtype=np.float64)
    rs_total = np.zeros(B, dtype=np.float64)
    for c, r in enumerate(res.results):
        rb, ch = c // 2, c % 2
        cs = r["colsum"].astype(np.float64)
        # cs[:, tc] are RUNNING column sums; telescoping differences
        run = cs[:, :CT]
        per_tile = np.diff(
            np.concatenate([np.zeros((P, 1)), run], axis=1), axis=1
        )
        # per_tile[p, tc] covers global col  ch*CB + tc*128 + p
        cs_total[ch * CB : (ch + 1) * CB] += per_tile.T.reshape(CB)
        dtot += np.float64(cs[:, CT].sum())
        rs_total[rb * RB : (rb + 1) * RB] += (
            r["rowsum"].reshape(RB).astype(np.float64)
        )
    total = dtot + np.log(rs_total).sum() + np.log(cs_total).sum()
    loss = total / (2 * B)
    return np.array(loss, dtype=np.float32)
